# revision 32
# baseline (speedup 1.0000x reference)
"""MANN (phase-blended mixture-of-experts) forward pass on 8 Trainium2 cores.

Strategy (data-parallel, per sharding hint):
  - Shard batch B=512 across 8 cores (64 samples each); replicate all weights.
  - Weights are DMA'd to SBUF ONCE (bf16, ~12 MB resident) and reused by
    every kernel body: the marginal body does no weight DMA at all.
  - Matmul layout: weights are the 128x128 STATIONARY (full PE array), the
    per-expert scaled activations xk[e] = g[:,e] * x stream as the N=64
    moving operand.  Output lands in PSUM as [out_feat, batch] -- already
    transposed for the next layer, so there are no PE transposes at all;
    the final [OUT, B] output is un-transposed on the host.
        y^T = sum_e W_e^T-slab.T @ (g_e * x)^T   (+ bias via small matmul)
  - All elementwise work runs on [128, 64] tiles (full partition width):
    ELU = max(z, min(exp(z),1)-1) split as exp on ACT, min/add on GPSIMD,
    max on DVE (which also serves as the PSUM->SBUF eviction); the g-scaling
    for the next layer is one broadcast tensor_tensor per output tile.
  - OUT_DIM padded 400->512 host-side so all layers are uniform 4x128
    output slices (the pad columns multiply zero weights).
"""

import json
import os

import numpy as np
import ml_dtypes

import concourse.bass as bass
import concourse.bass2jax as bass2jax
import concourse.mybir as mybir
import concourse.tile as tile
from concourse import bass_utils as _bass_utils
from concourse.bass_utils import run_bass_kernel_spmd


def _legalize_bir(bir_bytes):
    """This container's walrus build rejects instructions carrying more than
    one semaphore wait (setupSyncWait: "Too many sync wait commands" -- hit by
    the Tile kernel-tail Drain).  Equivalent legal form: hoist all but one
    wait onto single-wait NoOps immediately preceding the instruction on the
    same engine (sequencers process waits in program order)."""
    data = json.loads(bir_bytes)
    n = 0
    for fn in data.get("functions", []):
        for bb in fn.get("blocks", []):
            out = []
            for inst in bb.get("instructions", []):
                si = inst.get("sync_info")
                waits = si.get("on_wait", []) if si else []
                if len(waits) > 1:
                    for w in waits[:-1]:
                        n += 1
                        out.append({
                            "debug": inst.get("debug", 0),
                            "engine": inst["engine"],
                            "ins": [], "outs": [],
                            "name": f"I-mwfix-{n}",
                            "opcode": "NoOp",
                            "sync_info": {"on_update": [], "on_wait": [w]},
                        })
                    si["on_wait"] = [waits[-1]]
                out.append(inst)
            bb["instructions"] = out
    return json.dumps(data).encode()


_orig_compile_bir_kernel = _bass_utils.compile_bir_kernel


def _patched_compile_bir_kernel(bir_json, tmpdir, neff_name="file.neff"):
    return _orig_compile_bir_kernel(_legalize_bir(bir_json), tmpdir,
                                    neff_name=neff_name)


bass2jax.compile_bir_kernel = _patched_compile_bir_kernel
_bass_utils.compile_bir_kernel = _patched_compile_bir_kernel

B, IN_DIM, OUT_DIM, HID, K, GH, NG = 512, 480, 400, 512, 8, 128, 32
N_CORES = 8
BS = B // N_CORES  # 64 samples per core
IN_PAD = 512       # layer-1 contraction dim padded to 4x128
OUT_PAD = 512      # layer-3 output dim padded to 4x128
KSUB = 4           # contraction subtiles (all layers, post-pad)
NT = 4             # output-feature tiles of 128 per layer
OUTS = (HID, HID, OUT_PAD)
P = 128

MM_MODE = os.environ.get("MANN_MM_MODE", "bf16")
WDT = os.environ.get("MANN_WDT", "bf16")  # bf16 | fp8e3 (motion weights)
WSCALE = 128.0

# Set to the BassKernelResults of the last run (for test harnesses).
LAST_RESULTS = None

_NC_CACHE = {}


def _build(mode, repeat=1):
    f32 = mybir.dt.float32
    bf16 = mybir.dt.bfloat16

    nc = bass.Bass()

    xT_d = nc.dram_tensor("xT", [IN_PAD, BS], bf16, kind="ExternalInput")
    ginT_d = nc.dram_tensor("ginT", [NG, BS], f32, kind="ExternalInput")
    wdt = mybir.dt.float8e3 if WDT == "fp8e3" else bf16
    w_d = [
        nc.dram_tensor(f"w{l}", [K, IN_PAD if l == 0 else HID, OUTS[l]],
                       wdt, kind="ExternalInput")
        for l in range(3)
    ]
    b_d = [
        nc.dram_tensor(f"b{l}", [K, OUTS[l]], bf16, kind="ExternalInput")
        for l in range(3)
    ]
    gw1_d = nc.dram_tensor("gw1", [NG, GH], f32, kind="ExternalInput")
    gw2_d = nc.dram_tensor("gw2", [GH, GH], f32, kind="ExternalInput")
    gw3_d = nc.dram_tensor("gw3", [GH, K], f32, kind="ExternalInput")
    gb1_d = nc.dram_tensor("gb1", [GH, 1], f32, kind="ExternalInput")
    gb2_d = nc.dram_tensor("gb2", [GH, 1], f32, kind="ExternalInput")
    gb3_d = nc.dram_tensor("gb3", [K, 1], f32, kind="ExternalInput")
    # E[j, e*128 + p] = (j == e): replicates g row e across 128 partitions
    # via matmul E_slice.T @ gT.
    emat_d = nc.dram_tensor("emat", [K, K * P], bf16, kind="ExternalInput")
    out_d = nc.dram_tensor("out", [OUT_PAD, BS], f32, kind="ExternalOutput")

    pb = os.environ.get("MANN_POOLS", "4.12.4").split(".")
    with tile.TileContext(nc) as tc:
        with (
            tc.tile_pool(name="consts", bufs=1) as cpool,
            tc.tile_pool(name="stat", bufs=int(pb[0])) as spool,
            tc.tile_pool(name="y", bufs=int(pb[1])) as ypool,
            tc.tile_pool(name="psy", bufs=int(pb[2]), space="PSUM") as pspool,
            tc.tile_pool(name="psg", bufs=1, space="PSUM") as pgpool,
        ):
            pools = (cpool, spool, ypool, pspool, pgpool)

            # ---- small consts first (gating can start as soon as they land)
            gin = cpool.tile([NG, BS], f32)
            nc.sync.dma_start(gin, ginT_d[:])
            gw1 = cpool.tile([NG, GH], f32)
            nc.sync.dma_start(gw1, gw1_d[:])
            gw2 = cpool.tile([GH, GH], f32)
            nc.sync.dma_start(gw2, gw2_d[:])
            gw3 = cpool.tile([GH, K], f32)
            nc.sync.dma_start(gw3, gw3_d[:])
            gb1 = cpool.tile([GH, 1], f32)
            nc.sync.dma_start(gb1, gb1_d[:])
            gb2 = cpool.tile([GH, 1], f32)
            nc.sync.dma_start(gb2, gb2_d[:])
            gb3 = cpool.tile([K, 1], f32)
            nc.sync.dma_start(gb3, gb3_d[:])
            emat = cpool.tile([K, K * P], bf16)
            nc.sync.dma_start(emat, emat_d[:])
            xt0 = cpool.tile([P, 1, KSUB, BS], bf16)
            nc.sync.dma_start(xt0[:, 0, :, :],
                              xT_d.rearrange("(ko p) b -> p ko b", p=P))
            bts = []
            for l in range(3):
                bt = cpool.tile([K, OUTS[l]], bf16, tag=f"b{l}")
                nc.sync.dma_start(bt, b_d[l][:])
                bts.append(bt)
            zeros = cpool.tile([P, 1, BS], f32)
            nc.vector.memset(zeros, 0.0)

            # ---- resident weight slabs: DMA'd once, reused every body ----
            wsl = []
            for l in range(3):
                row = []
                for e in range(K):
                    t = cpool.tile([P, KSUB, OUTS[l]], wdt, tag=f"w{l}_{e}")
                    nc.sync.dma_start(
                        t, w_d[l][e].rearrange("(ko p) n -> p ko n", p=P))
                    row.append(t)
                wsl.append(row)

            consts = (xt0, gin, gw1, gw2, gw3, gb1, gb2, gb3, emat, bts, wsl,
                      zeros)

            if repeat == 0:
                # no-op baseline for dispatch-overhead measurement
                yo = ypool.tile([P, NT, BS], f32, tag="yoT")
                nc.vector.memset(yo, 0.0)
                nc.sync.dma_start(
                    out_d.rearrange("(ot p) b -> p ot b", p=P), yo)
            for _rep in range(repeat):
                _emit_body(nc, pools, out_d, consts, accum=(_rep > 0))

    return nc


def _emit_body(nc, pools, out_d, consts, accum=False):
    f32 = mybir.dt.float32
    bf16 = mybir.dt.bfloat16
    cpool, spool, ypool, pspool, pgpool = pools
    (xt0, gin, gw1, gw2, gw3, gb1, gb2, gb3, emat, bts, wsl, zeros) = consts

    # PE-warmth fillers: the PE idles ~4us during the gating chain, long
    # enough for the HAM activity monitor to re-throttle it to 1.2 GHz (and
    # un-throttling needs 3.4us of sustained work, which the 1.8us matmul
    # blocks never provide).  Cheap independent matmuls on a scratch PSUM
    # bank keep the activity window busy so the real matmuls run at 2.4 GHz.
    n_fill = int(os.environ.get("MANN_FILL", "48"))
    elu_mode = os.environ.get("MANN_ELU", "max")
    post_gran = 1 if os.environ.get("MANN_POST", "ot") == "ot" else 2
    xk_gran = os.environ.get("MANN_XK", "ot")  # pair | ot
    n_fill_l = int(os.environ.get("MANN_FILL_L", "0"))
    fill_ps = pgpool.tile([P, 256], mybir.dt.float32, tag="fill")

    def fillers(n):
        for _ in range(n):
            nc.tensor.matmul(fill_ps, lhsT=emat[:, 0:P], rhs=emat[:, 0:256],
                             start=True, stop=True, skip_group_check=True)

    def elu_tail(dst, z_ap, texp_ap, zb):
        """dst = elu(z) given texp = exp(z), both [P, n, BS]-shaped APs.
        Two DVE ops streamed back-to-back (no cross-engine hop):
        v = min(texp - 1, 0); dst = max(z, v)."""
        nc.vector.scalar_tensor_tensor(texp_ap, texp_ap, -1.0, zb,
                                       mybir.AluOpType.add,
                                       mybir.AluOpType.min)
        nc.vector.tensor_tensor(dst, z_ap, texp_ap, mybir.AluOpType.max)

    # ---- gating MLP (fp32, exact) ----
    # ELU(z) with z = pg + b: exp(pg + b) on ACT (bias fused) in parallel
    # with the bias-add on DVE; tail on DVE.
    pg1 = pgpool.tile([GH, BS], f32, tag="psg")
    nc.tensor.matmul(pg1, lhsT=gw1, rhs=gin, start=True, stop=True)
    fillers(n_fill // 3)
    tex = ypool.tile([GH, BS], f32, tag="g1e")
    nc.scalar.activation(tex, pg1, mybir.ActivationFunctionType.Exp,
                         bias=gb1)
    zg1 = ypool.tile([GH, BS], f32, tag="zg1")
    nc.vector.tensor_scalar(zg1, pg1, gb1, 0.0, mybir.AluOpType.add,
                            mybir.AluOpType.add)
    h1 = ypool.tile([GH, BS], f32, tag="g1y")
    elu_tail(h1, zg1, tex, zeros[:, 0, :].to_broadcast((GH, BS)))

    pg2 = pgpool.tile([GH, BS], f32, tag="psg")
    nc.tensor.matmul(pg2, lhsT=gw2, rhs=h1, start=True, stop=True)
    fillers(n_fill // 3)
    tex2 = ypool.tile([GH, BS], f32, tag="g2e")
    nc.scalar.activation(tex2, pg2, mybir.ActivationFunctionType.Exp,
                         bias=gb2)
    zg2 = ypool.tile([GH, BS], f32, tag="zg2")
    nc.vector.tensor_scalar(zg2, pg2, gb2, 0.0, mybir.AluOpType.add,
                            mybir.AluOpType.add)
    h2 = ypool.tile([GH, BS], f32, tag="g2y")
    elu_tail(h2, zg2, tex2, zeros[:, 0, :].to_broadcast((GH, BS)))

    pg3 = pgpool.tile([K, BS], f32, tag="psg")
    nc.tensor.matmul(pg3, lhsT=gw3, rhs=h2, start=True, stop=True)
    # g in bf16 from here on (used as matmul operand and for scaling)
    gT_mm = ypool.tile([K, BS], bf16, tag="gTmm")
    nc.scalar.activation(gT_mm, pg3, mybir.ActivationFunctionType.Identity,
                         bias=gb3)

    # replicate g across partitions: gTb[p, e, 0, b] = g[b, e]  (bf16)
    pgt = pgpool.tile([P, K, BS], f32, tag="psgtb")
    for e in range(K):
        nc.tensor.matmul(pgt[:, e, :], lhsT=emat[:, e * P:(e + 1) * P],
                         rhs=gT_mm, start=True, stop=True)
    fillers(n_fill - 2 * (n_fill // 3))
    gTb = ypool.tile([P, K, 1, BS], bf16, tag="gTb")
    nc.scalar.activation(gTb[:, :, 0, :], pgt,
                         mybir.ActivationFunctionType.Identity,
                         scale=(1.0 / WSCALE) if WDT == "fp8e3" else 1.0)

    def xk_mults(dst_sl, src):
        """Write xk[:, :, ks-pair, :] = src * gTb; src is [P, 1, 2, BS]."""
        nc.vector.tensor_tensor(
            dst_sl, src.to_broadcast((P, K, 2, BS)),
            gTb.to_broadcast((P, K, 2, BS)),
            mybir.AluOpType.mult)

    # layer-0 scaled stationaries: xk[p, e, ks, b] = x^T * g_e (per-subtile
    # so the first matmul group can start after the first slice)
    xk = spool.tile([P, K, KSUB, BS], bf16, tag="xk")
    if os.environ.get("MANN_XK0", "ot") == "ot":
        for ks in range(KSUB):
            sl = slice(ks, ks + 1)
            nc.vector.tensor_tensor(
                xk[:, :, sl, :],
                xt0[:, :, sl, :].to_broadcast((P, K, 1, BS)),
                gTb.to_broadcast((P, K, 1, BS)), mybir.AluOpType.mult)
    else:
        for oh in range(2):
            sl = slice(2 * oh, 2 * oh + 2)
            xk_mults(xk[:, :, sl, :], xt0[:, :, sl, :])

    # ---- motion layers ----
    # psO[ot] accumulates y^T[o_slice, b] over bias + all (e, ks); the
    # post-processing of tile ot overlaps the matmuls of tile ot+1.
    for l in range(3):
        psO = pspool.tile([P, NT, BS], f32, tag="psO", name=f"psO{l}")
        if l < 2:
            xk_next = spool.tile([P, K, KSUB, BS], bf16, tag="xk")
        else:
            yoT = ypool.tile([P, NT, BS], f32, tag="yoT")
        # Wavefront emission: all matmul groups first, then each post stage
        # across the 4 output tiles, so every engine queue is a stream of
        # independent ops (strict-FIFO queues; no head-of-line blocking).
        for ot in range(NT):
            lo = ot * P
            ps = psO[:, ot, :]
            nc.tensor.matmul(ps, lhsT=bts[l][:, lo:lo + P], rhs=gT_mm,
                             start=True, stop=False, skip_group_check=True)
            for ks in range(KSUB):
                for e in range(K):
                    nc.tensor.matmul(
                        ps,
                        lhsT=wsl[l][e][:, ks, lo:lo + P],
                        rhs=xk[:, e, ks, :],
                        start=False,
                        stop=(ks == KSUB - 1 and e == K - 1),
                        skip_group_check=True,
                    )
        fillers(n_fill_l)
        if l < 2:
            # ELU on [feat, b] tiles, then the g-scale for the next layer.
            # mode "max":  exp on ACT; min/add + max on DVE (max doubles as
            #              the PSUM->SBUF eviction, 1x due to PSUM operand).
            # mode "relu": elu(z) = relu(z) - relu(1 - exp(z)) -- three ACT
            #              ops per slice but only ONE DVE op, a bf16 SBUF
            #              subtract that runs in the 2x perf mode.
            texp = ypool.tile([P, NT, BS], f32, tag="te")
            yf = ypool.tile([P, 1, NT, BS], bf16, tag="yf")
            if elu_mode == "relu":
                r1 = ypool.tile([P, NT, BS], bf16, tag="r1")
                r2 = ypool.tile([P, NT, BS], bf16, tag="r2")
                for oh in range(2):
                    sl = slice(2 * oh, 2 * oh + 2)
                    nc.scalar.activation(texp[:, sl, :], psO[:, sl, :],
                                         mybir.ActivationFunctionType.Exp)
                for oh in range(2):
                    sl = slice(2 * oh, 2 * oh + 2)
                    nc.scalar.activation(r1[:, sl, :], psO[:, sl, :],
                                         mybir.ActivationFunctionType.Relu)
                for oh in range(2):
                    sl = slice(2 * oh, 2 * oh + 2)
                    nc.scalar.activation(r2[:, sl, :], texp[:, sl, :],
                                         mybir.ActivationFunctionType.Relu,
                                         bias=1.0, scale=-1.0)
                for oh in range(2):
                    sl = slice(2 * oh, 2 * oh + 2)
                    nc.vector.tensor_tensor(yf[:, 0, sl, :], r1[:, sl, :],
                                            r2[:, sl, :],
                                            mybir.AluOpType.subtract)
            else:
                g = post_gran
                for oh in range(NT // g):
                    sl = slice(g * oh, g * oh + g)
                    nc.scalar.activation(texp[:, sl, :], psO[:, sl, :],
                                         mybir.ActivationFunctionType.Exp)
                if os.environ.get("MANN_DVEORD", "ot") == "ot":
                    # ot-major DVE order: the next layer's first xk slice
                    # completes after 3 DVE ops instead of 9 (exps on ACT
                    # finish early, so no head-of-line blocking).
                    for ot in range(NT):
                        sl = slice(ot, ot + 1)
                        elu_tail(yf[:, 0, sl, :], psO[:, sl, :],
                                 texp[:, sl, :],
                                 zeros.to_broadcast((P, 1, BS)))
                        nc.vector.tensor_tensor(
                            xk_next[:, :, sl, :],
                            yf[:, :, sl, :].to_broadcast((P, K, 1, BS)),
                            gTb.to_broadcast((P, K, 1, BS)),
                            mybir.AluOpType.mult)
                else:
                    for oh in range(NT // g):
                        sl = slice(g * oh, g * oh + g)
                        elu_tail(yf[:, 0, sl, :], psO[:, sl, :],
                                 texp[:, sl, :],
                                 zeros.to_broadcast((P, g, BS)))
                    if xk_gran == "ot":
                        for ot in range(NT):
                            sl = slice(ot, ot + 1)
                            nc.vector.tensor_tensor(
                                xk_next[:, :, sl, :],
                                yf[:, :, sl, :].to_broadcast((P, K, 1, BS)),
                                gTb.to_broadcast((P, K, 1, BS)),
                                mybir.AluOpType.mult)
                    else:
                        for oh in range(2):
                            sl = slice(2 * oh, 2 * oh + 2)
                            xk_mults(xk_next[:, :, sl, :], yf[:, :, sl, :])
            xk = xk_next
        else:
            nc.scalar.activation(yoT, psO,
                                 mybir.ActivationFunctionType.Identity)
            if accum:
                # benchmark-repeat builds accumulate so no body is dead code
                nc.gpsimd.dma_start(
                    out_d.rearrange("(ot p) b -> p ot b", p=P), yoT,
                    accum_op=mybir.AluOpType.add)
            else:
                nc.sync.dma_start(
                    out_d.rearrange("(ot p) b -> p ot b", p=P), yoT)


def _get_nc(mode):
    repeat = int(os.environ.get("MANN_BENCH_REPEAT", "1"))
    key = (mode, repeat)
    if key not in _NC_CACHE:
        _NC_CACHE[key] = _build(mode, repeat)
    return _NC_CACHE[key]


def _make_emat():
    e = np.zeros((K, K * P), np.float32)
    for j in range(K):
        e[j, j * P:(j + 1) * P] = 1.0
    return e


def prepare_inputs(x, gating_idx, GW1, Gb1, GW2, Gb2, GW3, Gb3,
                   Wk1, bk1, Wk2, bk2, Wk3, bk3, mode):
    wnp = ml_dtypes.bfloat16
    f32 = np.float32
    x = np.asarray(x, f32)
    idx = np.asarray(gating_idx).astype(np.int64)

    xT = np.zeros((IN_PAD, B), f32)
    xT[:IN_DIM] = x.T
    ginT = np.ascontiguousarray(x[:, idx].T)

    w1 = np.zeros((K, IN_PAD, HID), f32)
    w1[:, :IN_DIM] = np.asarray(Wk1, f32).transpose(0, 2, 1)
    w2 = np.ascontiguousarray(np.asarray(Wk2, f32).transpose(0, 2, 1))
    w3 = np.zeros((K, HID, OUT_PAD), f32)
    w3[:, :, :OUT_DIM] = np.asarray(Wk3, f32).transpose(0, 2, 1)
    b3 = np.zeros((K, OUT_PAD), f32)
    b3[:, :OUT_DIM] = np.asarray(bk3, f32)

    if WDT == "fp8e3":
        wq = ml_dtypes.float8_e3m4
        wvals = (np.clip(w1 * WSCALE, -15.5, 15.5).astype(wq),
                 np.clip(w2 * WSCALE, -15.5, 15.5).astype(wq),
                 np.clip(w3 * WSCALE, -15.5, 15.5).astype(wq))
    else:
        wvals = (w1.astype(wnp), w2.astype(wnp), w3.astype(wnp))
    shared = {
        "w0": wvals[0], "w1": wvals[1], "w2": wvals[2],
        "b0": np.asarray(bk1, f32).astype(wnp),
        "b1": np.asarray(bk2, f32).astype(wnp),
        "b2": b3.astype(wnp),
        "gw1": np.asarray(GW1, f32), "gw2": np.asarray(GW2, f32),
        "gw3": np.asarray(GW3, f32),
        "gb1": np.asarray(Gb1, f32).reshape(GH, 1),
        "gb2": np.asarray(Gb2, f32).reshape(GH, 1),
        "gb3": np.asarray(Gb3, f32).reshape(K, 1),
        "emat": _make_emat().astype(wnp),
    }
    xTb = xT.astype(wnp)
    in_maps = []
    for c in range(N_CORES):
        m = dict(shared)
        m["xT"] = np.ascontiguousarray(xTb[:, c * BS:(c + 1) * BS])
        m["ginT"] = np.ascontiguousarray(ginT[:, c * BS:(c + 1) * BS])
        in_maps.append(m)
    return in_maps


def kernel(**inputs):
    global LAST_RESULTS
    mode = MM_MODE
    nc = _get_nc(mode)
    in_maps = prepare_inputs(mode=mode, **inputs)
    trace = os.environ.get("MANN_TRACE", "0") == "1"
    kwargs = {}
    if trace:
        kwargs["trace"] = True
    # Axon executions very occasionally return a corrupted buffer (observed
    # ~1/20 as inf-filled output with the same NEFF passing 30/30 on rerun);
    # retry on non-finite output as cheap insurance.
    for attempt in range(3):
        res = run_bass_kernel_spmd(nc, in_maps,
                                   core_ids=list(range(N_CORES)), **kwargs)
        LAST_RESULTS = res
        out = np.concatenate(
            [r["out"][:OUT_DIM].T for r in res.results], axis=0)
        if np.isfinite(out).all():
            break
    return np.ascontiguousarray(out, np.float32)
